# revision 1
# baseline (speedup 1.0000x reference)
"""Self-contained TRN2 Bass kernel for nn_AE_37117107372093 (tree-merge GNN).

kernel(**inputs) -> np.ndarray [4194304, 16] float32.

Strategy: merges sharded M/8 per core per level; all static (initial
Feature/X) side reads are host-resolved into per-core per-level slot
streams (indices are compile-time constants); dynamic reads come from a
DRAM arena via per-partition indirect DMAs; per-level AllToAll broadcasts
the prefix of each core's output block that any peer will later read.
MLP runs as block-diagonal bf16 matmuls on the TensorEngine with DVE
32x32 stream transposes; left+right contributions are summed in PSUM.
"""
"""Distributed TRN2 kernel for the tree-merge GNN (nn_AE_37117107372093).

Design (v2 — works around broken multi-index indirect DMA)
----------------------------------------------------------
* All feature data bf16. Gather slots are 32 bf16 (64B): [F(16) | X(5) | pad].
* Host pre-resolves every STATIC side read (initial Feature/X — indices are
  compile-time constants) into per-(core, level) slot streams `gstream`;
  one plain DMA fills the whole gather tile G per level. Dynamic slots get
  [0(16) | X(5) | 0] from the stream; their F part is patched by
  [P,1]-indexed indirect DMAs (one index per partition — the only reliable
  form on this stack) from the DRAM arena.
* Merges are sorted so dynamic sides pack into full 128-row j-columns.
* Compute: DVE 32x32 stream-transpose -> feature-major; 3 block-diagonal
  bf16 matmuls per 512-wide window (4 blocks); right-side layer 3
  accumulates into the left PSUM (out = mlp(l)+mlp(r) for free); scalar
  engine applies fused biases (layer-1 bias folds in the level positional
  encoding); back-transpose; DMA to the arena O region (= output).
* Comm: arena O_i is ordered so rows needed by ANY peer form the prefix
  [0, U_i). AllToAll broadcasts the prefix (same shard to all peers, via a
  stride-0 broadcast AP); receivers index the recv region directly.
  No staging gathers.
"""
from dataclasses import dataclass, field

import numpy as np

N_FEAT = 16
POS_DIM = 5
IN_CH = 21
SLOT = 32           # bf16 elems per gather slot (64B)
P = 128             # partitions
WCOLS = 16          # j-cols per matmul window (window free = WCOLS*32 = 512)


def to_bf16(x):
    u = np.ascontiguousarray(x, np.float32).view(np.uint32)
    r = ((u >> 16) & 1) + 0x7FFF
    return ((u + r) & 0xFFFF0000).view(np.float32)


def sinusoid_table(L, d):
    pos = np.arange(L, dtype=np.float32)[:, None]
    j = np.arange(d)
    angle = pos / np.power(10000.0, (2 * (j // 2)).astype(np.float32) / d)
    return np.where(j % 2 == 0, np.sin(angle), np.cos(angle)).astype(np.float32)


@dataclass
class Plan:
    n: int
    L: int
    M: int
    C: int
    mpc: int
    Jp: list = field(default_factory=list)     # padded j-cols per level
    W: list = field(default_factory=list)      # windows per level
    dyncols: list = field(default_factory=list)  # per level: dyn j-col indices in [0, 2*Jp)
    S: list = field(default_factory=list)      # A2A shard rows per level
    o_base: list = field(default_factory=list)
    r_base: list = field(default_factory=list)
    s_base: list = field(default_factory=list)
    n_arena: int = 0
    n_out: int = 0
    idxB: np.ndarray = None      # [C, L, P, DCmax] int32 arena rows for dyn cols
    out_node: np.ndarray = None  # [C, n_out] int64
    gs_node: np.ndarray = None   # [C, L, P, 2*Jmax] int64: node id static; -1-r dyn; -2 pad
    Jmax: int = 0
    DCmax: int = 0


def _window_bijection(Wn, Jp):
    """arena_row (level-local) -> flat merge id p*Jp + j (device S->B emulation)."""
    out = np.empty(Wn * 2048, dtype=np.int64)
    for w in range(Wn):
        S = np.empty((64, 512), dtype=np.int64)
        for b in range(4):
            for fo in range(16):
                for jj in range(16):
                    rp = np.arange(32)
                    S[16 * b + fo, 32 * jj + rp] = (32 * b + rp) * Jp + (16 * w + jj)
        B = np.empty_like(S)
        for a in range(0, 64, 32):
            for cblk in range(0, 512, 32):
                B[a:a + 32, cblk:cblk + 32] = S[a:a + 32, cblk:cblk + 32].T
        out[w * 2048:(w + 1) * 2048] = B[:, ::16].reshape(-1)
    return out


def make_plan(I_list, n, C=8):
    L, M, _ = I_list.shape
    mpc = M // C
    assert M % C == 0 and mpc % P == 0

    pl = Plan(n=n, L=L, M=M, C=C, mpc=mpc)
    li_all = I_list[:, :, 0].astype(np.int64)
    ri_all = I_list[:, :, 1].astype(np.int64)
    pi_all = I_list[:, :, 2].astype(np.int64)

    # ---------- need sets ---------------------------------------------------
    own_of_row = np.full(L * M, -1, dtype=np.int8)
    for i in range(L):
        own_of_row[pi_all[i]] = np.repeat(np.arange(C, dtype=np.int8), mpc)

    need_remote = [[set() for _ in range(C)] for _ in range(L)]
    for j in range(L):
        for c in range(C):
            sl = slice(c * mpc, (c + 1) * mpc)
            for vals in (li_all[j, sl], ri_all[j, sl]):
                dyn = vals < j * M
                rr = np.unique(vals[dyn])
                if rr.size:
                    own = own_of_row[rr]
                    for r in rr[own != c]:
                        need_remote[r // M][own_of_row[r]].add(int(r))

    # ---------- group ordering ----------------------------------------------
    # 9 subgroups: SD split by right-late (0,1), SS (2), DS split by left-late
    # (3,4), DD split by (left-late, right-late) (5..8). "late" = the side
    # reads a REMOTE row of level i-1 (needs the latest AllToAll).
    NG = 9
    order = [[None] * C for _ in range(L)]
    counts = np.zeros((L, C, NG), dtype=np.int64)
    for i in range(L):
        for c in range(C):
            sl = slice(c * mpc, (c + 1) * mpc)
            lv = li_all[i, sl]
            rv = ri_all[i, sl]
            ld = lv < i * M
            rd = rv < i * M
            llate = ld & (lv >= (i - 1) * M) & (own_of_row[np.minimum(lv, L * M - 1)] != c)
            rlate = rd & (rv >= (i - 1) * M) & (own_of_row[np.minimum(rv, L * M - 1)] != c)
            g = np.full(mpc, -1, dtype=np.int64)
            g[~ld & rd] = 0 + rlate[~ld & rd]
            g[~ld & ~rd] = 2
            g[ld & ~rd] = 3 + llate[ld & ~rd]
            dd = ld & rd
            g[dd] = 5 + 2 * llate[dd] + rlate[dd]
            order[i][c] = np.argsort(g, kind="stable")
            counts[i, c] = np.bincount(g, minlength=NG)
    bounds = []
    for i in range(L):
        t = np.ceil(counts[i].max(axis=0) / P).astype(np.int64)
        Jp = int(((t.sum() + WCOLS - 1) // WCOLS) * WCOLS)
        bounds.append(np.concatenate([[0], np.cumsum(t)]).astype(int))
        pl.Jp.append(Jp)
        pl.W.append(Jp // WCOLS)
    pl.Jmax = max(pl.Jp)

    o_rows = [pl.W[i] * 2048 for i in range(L)]
    pl.o_base = np.concatenate([[0], np.cumsum(o_rows)])[:L].tolist()
    pl.n_out = int(sum(o_rows))

    # dyn columns with late flags. left-dyn: keys 3..8; right-dyn: 0,1,5..8
    for i in range(L):
        b = bounds[i]
        Jp = pl.Jp[i]
        cols = []
        for k in (3, 4, 5, 6, 7, 8):           # left side dyn
            late = k == 4 or k >= 7
            cols += [(col, late) for col in range(b[k], b[k + 1])]
        for k in (0, 1, 5, 6, 7, 8):           # right side dyn
            late = k == 1 or k in (6, 8)
            cols += [(Jp + col, late) for col in range(b[k], b[k + 1])]
        pl.dyncols.append(cols)
    pl.DCmax = max((len(c) for c in pl.dyncols), default=0)

    # ---------- slot assignment + arena layout ------------------------------
    node_row = np.full(L * M, -1, dtype=np.int64)
    out_node = np.full((C, pl.n_out), -1, dtype=np.int64)
    gs_node = np.full((C, L, P, 2 * pl.Jmax), -2, dtype=np.int64)
    slot_of_merge = [[None] * C for _ in range(L)]
    for i in range(L):
        Jp = pl.Jp[i]
        bij = _window_bijection(pl.W[i], Jp)
        arow_of_slot = np.empty(P * Jp, dtype=np.int64)
        arow_of_slot[bij] = np.arange(bij.size)
        b = bounds[i]
        for c in range(C):
            o = order[i][c]
            sl = slice(c * mpc, (c + 1) * mpc)
            pi_s = pi_all[i, sl][o]
            rem_set = need_remote[i][c]
            is_rem = np.fromiter((int(r) in rem_set for r in pi_s),
                                 dtype=bool, count=mpc)
            slot_idx = np.empty(mpc, dtype=np.int64)
            pos = 0
            for gg in range(9):
                cnt = int(counts[i, c, gg])
                if cnt == 0:
                    continue
                cols = np.arange(b[gg], b[gg + 1])
                slots = (np.arange(P)[:, None] * Jp + cols[None, :]).reshape(-1)
                srt = slots[np.argsort(arow_of_slot[slots], kind="stable")]
                idxs = np.arange(pos, pos + cnt)
                r_mask = is_rem[idxs]
                nrem = int(r_mask.sum())
                sel = np.empty(cnt, dtype=np.int64)
                sel[r_mask] = srt[:nrem]
                sel[~r_mask] = srt[nrem:cnt]
                slot_idx[idxs] = sel
                pos += cnt
            ar = pl.o_base[i] + arow_of_slot[slot_idx]
            node_row[pi_s] = ar
            out_node[c, ar] = pi_s
            slot_of_merge[i][c] = slot_idx

    # ---------- A2A shard sizes (prefix covering all remote-needed rows) ----
    Spre = np.zeros((L, C), dtype=np.int64)
    for i in range(L):
        for c in range(C):
            rem_set = need_remote[i][c]
            if rem_set:
                rows = node_row[np.fromiter(rem_set, dtype=np.int64)]
                Spre[i, c] = int(rows.max()) - pl.o_base[i] + 1
    for i in range(L):
        s = int(Spre[i].max())
        pl.S.append(((s + 15) // 16) * 16 if s else 0)

    base = pl.n_out
    for i in range(L):
        pl.r_base.append(base)
        base += C * pl.S[i]
    pl.s_base = []
    for i in range(L):
        pl.s_base.append(base)
        base += C * pl.S[i]
    pl.n_arena = base

    # ---------- read index resolution ---------------------------------------
    idxB = np.zeros((C, L, P, max(pl.DCmax, 1)), dtype=np.int32)
    ob = np.asarray(pl.o_base + [0])
    rb = np.asarray(pl.r_base + [0])
    Sarr = np.asarray(pl.S + [0])
    for i in range(L):
        Jp = pl.Jp[i]
        colpos = {col: k for k, (col, _late) in enumerate(pl.dyncols[i])}
        for c in range(C):
            sl = slice(c * mpc, (c + 1) * mpc)
            o = order[i][c]
            slot_idx = slot_of_merge[i][c]
            p_s = slot_idx // Jp
            j_s = slot_idx % Jp
            for side, vals in ((0, li_all[i, sl][o]), (1, ri_all[i, sl][o])):
                jj = j_s + side * Jp
                dyn = vals < i * M
                gs_node[c, i, p_s[~dyn], jj[~dyn]] = vals[~dyn]
                if dyn.any():
                    rr = vals[dyn]
                    own = own_of_row[rr]
                    rows = node_row[rr].copy()
                    rem = own != c
                    if rem.any():
                        rr_r = rr[rem]
                        lv = rr_r // M
                        loc = node_row[rr_r] - ob[lv]
                        assert (loc < Sarr[lv]).all()
                        rows[rem] = rb[lv] + own[rem].astype(np.int64) * Sarr[lv] + loc
                    gs_node[c, i, p_s[dyn], jj[dyn]] = -1 - rr
                    for pp, jcol, arow in zip(p_s[dyn], jj[dyn], rows):
                        idxB[c, i, pp, colpos[jcol]] = arow
    pl.idxB = idxB
    pl.gs_node = gs_node
    pl.out_node = out_node
    return pl


def build_gstream(pl, tab32):
    """[C, L, P, 2*Jmax*SLOT] f32 slot stream (to be cast bf16)."""
    C, L = pl.C, pl.L
    gs = np.zeros((C, L, P, 2 * pl.Jmax, SLOT), dtype=np.float32)
    zf = np.zeros_like(tab32)
    zf[:, N_FEAT:] = tab32[:, N_FEAT:]
    for c in range(C):
        for i in range(L):
            sel = pl.gs_node[c, i]
            stat = sel >= 0
            dyn = sel <= -1
            pad = sel == -2
            dyn &= ~pad
            gs[c, i][stat] = tab32[sel[stat]]
            gs[c, i][dyn] = zf[(-1 - sel[dyn])]
    return gs.reshape(C, L, P, 2 * pl.Jmax * SLOT)


# ----------------------------------------------------------------------
# numpy end-to-end simulation (bf16 exact)
# ----------------------------------------------------------------------
def simulate_host(pl, X, Feature, ws):
    w1, b1, w2, b2, w3, b3 = ws
    L, C, M = pl.L, pl.C, pl.M
    pos = sinusoid_table(L, N_FEAT)
    w1b, w2b, w3b = to_bf16(w1), to_bf16(w2), to_bf16(w3)
    b1_lvl = (b1[None] + pos @ w1[:, POS_DIM:].T).astype(np.float32)

    tab32 = np.zeros((pl.n, SLOT), dtype=np.float32)
    tab32[:, :N_FEAT] = to_bf16(Feature)
    tab32[:, N_FEAT:N_FEAT + POS_DIM] = to_bf16(X)
    gstream = build_gstream(pl, tab32)

    arena = [np.zeros((pl.n_arena, N_FEAT), dtype=np.float32) for _ in range(C)]
    for i in range(L):
        Jp = pl.Jp[i]
        dc = pl.dyncols[i]
        for c in range(C):
            G = gstream[c, i].reshape(P, 2 * pl.Jmax, SLOT)[:, :2 * Jp].copy()
            for k, (col, _late) in enumerate(dc):
                G[:, col, :N_FEAT] = arena[c][pl.idxB[c, i, :, k]]
            V = G.reshape(P * 2 * Jp, SLOT)
            h = np.maximum(V[:, :N_FEAT] @ w1b[:, POS_DIM:].T
                           + V[:, N_FEAT:IN_CH] @ w1b[:, :POS_DIM].T
                           + b1_lvl[i], 0)
            h = to_bf16(h)
            h = to_bf16(np.maximum(h @ w2b.T + b2, 0))
            y = (h @ w3b.T).reshape(P, 2 * Jp, N_FEAT)
            out = to_bf16(y[:, :Jp] + y[:, Jp:] + 2 * b3)
            bij = _window_bijection(pl.W[i], Jp)
            flat = out.reshape(P * Jp, N_FEAT)
            arena[c][pl.o_base[i]:pl.o_base[i] + pl.W[i] * 2048] = flat[bij]
        Si = pl.S[i]
        if Si:
            for d in range(C):
                for s in range(C):
                    blk = arena[s][pl.o_base[i]:pl.o_base[i] + Si]
                    arena[d][pl.r_base[i] + s * Si:
                             pl.r_base[i] + (s + 1) * Si] = blk
    return [a[:pl.n_out].copy() for a in arena]


def assemble_output(pl, Feature, outs):
    F = Feature.astype(np.float32).copy()
    for c in range(pl.C):
        m = pl.out_node[c] >= 0
        F[pl.out_node[c][m]] = outs[c][m]
    return F


# ----------------------------------------------------------------------
# bass graph
# ----------------------------------------------------------------------
def build_weights(pl, ws):
    w1, b1, w2, b2, w3, b3 = ws
    pos = sinusoid_table(pl.L, N_FEAT)
    b1_lvl = (b1[None] + pos @ w1[:, POS_DIM:].T).astype(np.float32)
    W1 = np.zeros((P, 64), dtype=np.float32)
    W2 = np.zeros((64, 64), dtype=np.float32)
    W3 = np.zeros((64, 64), dtype=np.float32)
    for b in range(4):
        W1[32 * b:32 * b + 16, 16 * b:16 * b + 16] = w1[:, POS_DIM:].T
        W1[32 * b + 16:32 * b + 21, 16 * b:16 * b + 16] = w1[:, :POS_DIM].T
        W2[16 * b:16 * b + 16, 16 * b:16 * b + 16] = w2.T
        W3[16 * b:16 * b + 16, 16 * b:16 * b + 16] = w3.T
    wgt = np.zeros((P, 192), dtype=np.float32)
    wgt[:, :64] = W1
    wgt[:64, 64:128] = W2
    wgt[:64, 128:192] = W3
    bias = np.zeros((64, pl.L + 2), dtype=np.float32)
    for i in range(pl.L):
        bias[:, i] = np.tile(b1_lvl[i], 4)
    bias[:, pl.L] = np.tile(b2, 4)
    bias[:, pl.L + 1] = np.tile(2 * b3, 4)
    return wgt, bias


def build_nc(pl):
    import concourse.bass as bass
    import concourse.mybir as mybir
    import concourse.tile as tile
    from concourse import bacc

    bf = mybir.dt.bfloat16
    f32 = mybir.dt.float32
    i32 = mybir.dt.int32
    L = pl.L

    nc = bacc.Bacc("TRN2", target_bir_lowering=False, debug=False)
    gstream_p = nc.declare_dram_parameter(
        "gstream", [L, P, 2 * pl.Jmax * SLOT], bf, isOutput=False)
    idxB_p = nc.declare_dram_parameter(
        "idxB", [L, P, max(pl.DCmax, 1)], i32, isOutput=False)
    wgt_p = nc.declare_dram_parameter("wgt", [P, 192], bf, isOutput=False)
    bias_p = nc.declare_dram_parameter("bias", [64, L + 2], f32, isOutput=False)
    out_p = nc.declare_dram_parameter("out", [pl.n_out, N_FEAT], bf,
                                      isOutput=True)
    arena = nc.dram_tensor("arena", [pl.n_arena, N_FEAT], bf)

    rg = [list(range(pl.C))]

    with tile.TileContext(nc) as tc:
        with tc.tile_pool(name="const", bufs=1) as cpool, \
             tc.tile_pool(name="idx", bufs=4) as ipool, \
             tc.tile_pool(name="gath", bufs=3) as gpool, \
             tc.tile_pool(name="tr", bufs=3) as tpool, \
             tc.tile_pool(name="h", bufs=3) as hpool, \
             tc.tile_pool(name="sb", bufs=3) as sbpool, \
             tc.tile_pool(name="ps", bufs=2, space="PSUM") as pspool, \
             tc.tile_pool(name="psS", bufs=2, space="PSUM") as psSpool:

            wq = cpool.tile([P, 192], bf, name="wq")
            nc.sync.dma_start(out=wq[:], in_=wgt_p[:, :])
            bq = cpool.tile([64, L + 2], f32, name="bq")
            nc.sync.dma_start(out=bq[:], in_=bias_p[:, :])
            W1ap = wq[:, 0:64]
            W2ap = wq[0:64, 64:128]
            W3ap = wq[0:64, 128:192]

            for i in range(L):
                Jp = pl.Jp[i]
                Wn = pl.W[i]
                dc = pl.dyncols[i]
                Si = pl.S[i]

                G = gpool.tile([P, 2 * Jp * SLOT], bf, tag="G", name=f"G{i}")
                nc.sync.dma_start(out=G[:],
                                  in_=gstream_p[i, :, :2 * Jp * SLOT])

                if dc:
                    iB = ipool.tile([P, len(dc)], i32, tag="iB", name=f"iB{i}")
                    nc.sync.dma_start(out=iB[:], in_=idxB_p[i, :, :len(dc)])
                    early_end = pl.r_base[i - 1] if i >= 1 else pl.n_out
                    late_end = (pl.r_base[i - 1] + pl.C * pl.S[i - 1]
                                if i >= 1 else pl.n_out)
                    for k, (col, late) in enumerate(dc):
                        rng = late_end if late else early_end
                        nc.gpsimd.indirect_dma_start(
                            out=G[:, col * SLOT:col * SLOT + N_FEAT],
                            out_offset=None,
                            in_=arena.ap()[0:rng, :],
                            in_offset=bass.IndirectOffsetOnAxis(
                                ap=iB[:, k:k + 1], axis=0),
                        )

                TR = tpool.tile([P, 2 * Jp * SLOT], bf, tag="TR", name=f"TR{i}")
                nc.vector.transpose(out=TR[:], in_=G[:])

                for w in range(Wn):
                    ps1 = pspool.tile([64, 512], f32, tag="ps1",
                                      name=f"ps1_{i}_{w}")
                    ps2 = pspool.tile([64, 512], f32, tag="ps2",
                                      name=f"ps2_{i}_{w}")
                    psS = psSpool.tile([64, 512], f32, tag="psS",
                                       name=f"psS_{i}_{w}")
                    SW = sbpool.tile([64, 512], bf, tag="SW", name=f"SW{i}_{w}")
                    BW = sbpool.tile([64, 512], bf, tag="BW", name=f"BW{i}_{w}")
                    for side in (0, 1):
                        rhs = TR[:, (side * Jp + w * WCOLS) * SLOT:
                                 (side * Jp + (w + 1) * WCOLS) * SLOT]
                        h1 = hpool.tile([64, 512], bf, tag="h1",
                                        name=f"h1_{i}_{w}_{side}")
                        h2 = hpool.tile([64, 512], bf, tag="h2",
                                        name=f"h2_{i}_{w}_{side}")
                        nc.tensor.matmul(out=ps1[:], lhsT=W1ap, rhs=rhs,
                                         start=True, stop=True)
                        nc.scalar.activation(
                            out=h1[:], in_=ps1[:],
                            func=mybir.ActivationFunctionType.Relu,
                            bias=bq[:, i:i + 1])
                        nc.tensor.matmul(out=ps2[:], lhsT=W2ap, rhs=h1[:],
                                         start=True, stop=True)
                        nc.scalar.activation(
                            out=h2[:], in_=ps2[:],
                            func=mybir.ActivationFunctionType.Relu,
                            bias=bq[:, L:L + 1])
                        nc.tensor.matmul(out=psS[:], lhsT=W3ap, rhs=h2[:],
                                         start=(side == 0), stop=(side == 1))
                    nc.scalar.activation(
                        out=SW[:], in_=psS[:],
                        func=mybir.ActivationFunctionType.Identity,
                        bias=bq[:, L + 1:L + 2])
                    nc.vector.transpose(out=BW[:], in_=SW[:])
                    nc.sync.dma_start(
                        out=arena.ap()[pl.o_base[i] + w * 2048:
                                       pl.o_base[i] + (w + 1) * 2048, :],
                        in_=BW[:])

                if Si > 0:
                    src = arena.ap()[pl.o_base[i]:pl.o_base[i] + Si, :]
                    src_b = bass.AP(src.tensor, src.offset,
                                    [[0, pl.C]] + [list(x) for x in src.ap])
                    send = arena.ap()[pl.s_base[i]:pl.s_base[i] + pl.C * Si, :]
                    nc.sync.dma_start(out=send, in_=src_b)
                    nc.gpsimd.collective_compute(
                        "AllToAll",
                        mybir.AluOpType.bypass,
                        replica_groups=rg,
                        ins=[send],
                        outs=[arena.ap()[pl.r_base[i]:
                                         pl.r_base[i] + pl.C * Si, :]],
                    )

            nc.sync.dma_start(out=out_p[:, :], in_=arena.ap()[0:pl.n_out, :])
    nc.compile()
    return nc


def kernel_run(X, Feature, I_list, ws, pl=None, trace=False):
    import ml_dtypes
    from concourse.bass_utils import run_bass_kernel_spmd

    if pl is None:
        pl = make_plan(I_list, n=X.shape[0])
    nc = build_nc(pl)
    wgt, bias = build_weights(pl, ws)

    tab32 = np.zeros((pl.n, SLOT), dtype=np.float32)
    tab32[:, :N_FEAT] = to_bf16(Feature)
    tab32[:, N_FEAT:N_FEAT + POS_DIM] = to_bf16(X)
    gstream = build_gstream(pl, tab32)

    in_maps = []
    for c in range(pl.C):
        in_maps.append({
            "gstream": gstream[c].astype(ml_dtypes.bfloat16),
            "idxB": pl.idxB[c],
            "wgt": wgt.astype(ml_dtypes.bfloat16),
            "bias": bias,
        })
    res = run_bass_kernel_spmd(nc, in_maps, core_ids=list(range(pl.C)),
                               trace=trace)
    outs = [np.asarray(r["out"]).astype(np.float32) for r in res.results]
    F = assemble_output(pl, Feature, outs)
    return F, res


_RUN_CACHE = {}


def kernel(**inputs):
    import time
    X = np.asarray(inputs["X"], dtype=np.float32)
    Feature = np.asarray(inputs["Feature"], dtype=np.float32)
    I_list = np.asarray(inputs["I_list"])
    ws = tuple(np.asarray(inputs[k], dtype=np.float32)
               for k in ("w1", "b1", "w2", "b2", "w3", "b3"))
    pl = make_plan(I_list, n=X.shape[0])
    last = None
    for attempt in range(2):
        try:
            F, _res = kernel_run(X, Feature, I_list, ws, pl=pl, trace=False)
            return F
        except Exception as e:  # transient device wedge: wait + retry once
            last = e
            time.sleep(240)
    raise last

